# revision 2
# baseline (speedup 1.0000x reference)
"""Trainium2 Bass kernel v3 for per-pixel (untied) local depthwise conv.

Problem: out[n,h,w,c] = sum_{dh,dw} in[n, h+dh-2, w+dw-2, c] * wt[n, h, w, dh*5+dw]
Shapes: in (8,512,512,3) f32, wt (8,512,512,25) f32, 'same' zero padding.

Design (one image per core, 8 cores):
  - The dw (column) shift is baked into per-tap weight planes on the HOST
    (content shifted, zero padded), so every DVE/Pool operand reads at its
    natural 4B-aligned position - no parity duplication of x.  The shift
    reappears as a column offset on the PE moving read, where it is free.
  - x stored once per core: [p, rr(8 halo rows), c, j(520)], 3.2MB fp16.
  - Output produced in TWO ROW-PASSES (rp = row-pair 2rp,2rp+1 of each
    partition): psum tile = 6 banks, bank = (c, rr) holding one full
    512-wide output row; each accumulation matmul is a contiguous 512-elem
    moving slice P[c, rr, dw:dw+512].
  - Products per (rp, tap): DVE tensor_mul [c,2,520] (TT is capped at 2x
    and 3 free dims on TRN2).  dh=1..4 on DVE, dh=0 on GPSIMD/Pool in
    parallel (psum accumulation order: dh 1,2,3,4 then 0).
  - Identity stationary loaded once: ldweights=False on all later matmuls.
  - Weight plane halves stream in consumption order; ~100KB SBUF total.
"""

import sys

sys.path.insert(0, "/opt/trn_rl_repo")

import numpy as np

import concourse.bass as bass
import concourse.mybir as mybir
from concourse.tile import TileContext
from concourse.bass_utils import run_bass_kernel_spmd

N, H, W, C, K = 8, 512, 512, 3, 5
KK = K * K
N_CORES = 8
RPP = 4                  # output rows per partition
HROWS = RPP + K - 1      # halo rows stored per partition (8)
JW = 520                 # padded row width (cols -2..517 at j-2)
X_FREE = HROWS * C * JW          # 12480 fp16 elems per partition
WT_HALF = 2 * JW                 # 1040 elems per (rp, tap) weight chunk
WT_FREE = 2 * KK * WT_HALF       # 52000
P_FREE = C * 2 * JW              # 3120 elems per (rp, tap) product
O_FREE = C * 2 * W               # 3072 out elems per partition per pass

USE_POOL = True          # dh=0 products on GPSIMD
SKIP_LDW = True          # ldweights=False on repeat identity matmuls


def _split_multi_waits(nc):
    """This walrus build encodes at most ONE sync-wait per instruction;
    hoist extra waits onto single-wait NOPs on the same engine."""
    n_split = 0
    for f in nc.m.functions:
        for bb in f.blocks:
            new_insts = []
            changed = False
            for inst in bb.instructions:
                si = inst.sync_info
                waits = list(si.on_wait) if (si is not None and si.on_wait) else []
                if len(waits) > 1:
                    changed = True
                    for w in waits[:-1]:
                        nop = mybir.InstNoOp(
                            name=nc.get_next_instruction_name(),
                            engine=inst.engine,
                            sync_info=mybir.SyncInfo(on_wait=[w], on_update=[]),
                            bass_nofuse=True,
                        )
                        new_insts.append(nop)
                        n_split += 1
                    inst.sync_info = mybir.SyncInfo(
                        on_wait=[waits[-1]],
                        on_update=list(si.on_update) if si.on_update else [],
                    )
                new_insts.append(inst)
            if changed:
                bb.instructions = new_insts
    return n_split


_NC_CACHE = None


def _build_program():
    global _NC_CACHE
    if _NC_CACHE is not None:
        return _NC_CACHE

    fp16 = mybir.dt.float16
    f32 = mybir.dt.float32

    nc = bass.Bass("TRN2", target_bir_lowering=False, debug=False,
                   num_devices=N_CORES)
    xbuf = nc.dram_tensor("xbuf", [128, X_FREE], fp16, kind="ExternalInput").ap()
    # wtbuf[p, rp, k, rr, j']  (natural k order; dh0 first = Pool's units)
    wtbuf = nc.dram_tensor("wtbuf", [128, WT_FREE], fp16,
                           kind="ExternalInput").ap()
    ident = nc.dram_tensor("ident", [128, 128], fp16, kind="ExternalInput").ap()
    out = nc.dram_tensor("out", [2, 128, O_FREE], fp16,
                         kind="ExternalOutput").ap()

    mms = []
    with TileContext(nc) as tc:
        with (
            tc.tile_pool(name="xpool", bufs=1) as xpool,
            tc.tile_pool(name="wtpool", bufs=12) as wtpool,
            tc.tile_pool(name="ppool", bufs=5) as ppool,
            tc.tile_pool(name="qpool", bufs=10) as qpool,
            tc.tile_pool(name="outpool", bufs=2) as outpool,
            tc.tile_pool(name="psumpool", bufs=1, space="PSUM") as psumpool,
        ):
            id_t = xpool.tile([128, 128], fp16)
            nc.sync.dma_start(out=id_t[:], in_=ident[:])

            x_t = xpool.tile([128, X_FREE], fp16)
            XR = C * JW
            # pass 0 touches rows 0..5, pass 1 rows 2..7
            nc.sync.dma_start(out=x_t[:, 0:3 * XR], in_=xbuf[:, 0:3 * XR])
            nc.scalar.dma_start(out=x_t[:, 3 * XR:6 * XR],
                                in_=xbuf[:, 3 * XR:6 * XR])
            nc.sync.dma_start(out=x_t[:, 6 * XR:], in_=xbuf[:, 6 * XR:])

            xv = x_t[:].rearrange("p (r c j) -> p r c j", r=HROWS, c=C)

            # stream weight chunks in consumption order: (rp, k) natural
            wt_tiles = {}
            for u in range(2 * KK):
                rp, k = u // KK, u % KK
                wtile = wtpool.tile([128, WT_HALF], fp16, tag="wt",
                                    name=f"wt_{rp}_{k}")
                eng = nc.sync if u % 2 == 0 else nc.scalar
                eng.dma_start(out=wtile[:],
                              in_=wtbuf[:, u * WT_HALF:(u + 1) * WT_HALF])
                wt_tiles[(rp, k)] = wtile

            def make_unit(rp, dh, dw):
                k = dh * K + dw
                pool = qpool if (USE_POOL and dh == 0) else ppool
                eng = nc.gpsimd if (USE_POOL and dh == 0) else nc.vector
                p_t = pool.tile([128, P_FREE], fp16, tag="p",
                                name=f"p_{rp}_{k}")
                pv = p_t[:].rearrange("p (c r j) -> p c r j", c=C, r=2)
                r0 = dh + 2 * rp
                xs = xv[:, r0:r0 + 2, :, :].transpose([0, 2, 1, 3])
                wk = (wt_tiles[(rp, k)][:]
                      .rearrange("p (r j) -> p r j", r=2)[:, None, :, :]
                      .broadcast_to([128, C, 2, JW]))
                eng.tensor_mul(out=pv, in0=xs, in1=wk)
                return pv

            DH_ORDER = [1, 2, 3, 4, 0] if USE_POOL else [0, 1, 2, 3, 4]

            for rp in range(2):
                ps_t = psumpool.tile([128, 6 * 512], f32, tag="ps",
                                     name=f"ps_{rp}")
                units = {}
                if USE_POOL:
                    for dw in range(K):     # Pool starts its slow units first
                        units[(0, dw)] = make_unit(rp, 0, dw)
                for di, dh in enumerate(DH_ORDER):
                    first_dh = di == 0
                    last_dh = di == K - 1
                    for dw in range(K):
                        if (dh, dw) not in units:
                            units[(dh, dw)] = make_unit(rp, dh, dw)
                        pv = units[(dh, dw)]
                        for c in range(C):
                            for rr in range(2):
                                bank = c * 2 + rr
                                mm = nc.tensor.matmul(
                                    ps_t[:, bank * 512:(bank + 1) * 512],
                                    id_t[:],
                                    pv[:, c, rr, dw:dw + W],
                                    start=(first_dh and dw == 0),
                                    stop=(last_dh and dw == K - 1),
                                )
                                mms.append(mm)

                o_t = outpool.tile([128, O_FREE], fp16)
                nc.scalar.copy(out=o_t[:], in_=ps_t[:])
                eng = nc.sync if rp == 0 else nc.scalar
                eng.dma_start(out=out[rp], in_=o_t[:])

    if SKIP_LDW:
        for i, mm in enumerate(mms):
            if i == 0:
                continue
            inst = getattr(mm, "inst", mm)
            inst.ldweights = False

    _split_multi_waits(nc)
    _NC_CACHE = nc
    return nc


def _pack_inputs(input_data: np.ndarray, weights: np.ndarray):
    """Host-side layout + fp16 conversion into per-core SBUF-ready buffers."""
    xh = input_data.astype(np.float16)     # (N, H, W, C)
    wh = weights.astype(np.float16)        # (N, H, W, KK)

    identity = np.eye(128, dtype=np.float16)
    in_maps = []
    for n in range(N_CORES):
        # x: [p, rr, c, j]: image row 4p+rr-2, col j-2 (zero pad)
        canvas = np.zeros((C, H + 4, JW), dtype=np.float16)
        canvas[:, 2:2 + H, 2:2 + W] = xh[n].transpose(2, 0, 1)
        sw = np.lib.stride_tricks.sliding_window_view(canvas, HROWS, axis=1)
        sw = sw[:, ::RPP][:, :128]            # (C, 128, JW, 8)
        X = np.ascontiguousarray(
            sw.transpose(1, 3, 0, 2).reshape(128, X_FREE))

        # weight chunks: wtbuf[p, rp, k, rr, j'] = wt[4p+2rp+rr, j'-dw, k]
        wtpad = np.zeros((H, JW, KK), dtype=np.float16)
        for dw in range(K):
            wtpad[:, dw:dw + W, dw::K] = wh[n][:, :, dw::K]
        WT = np.ascontiguousarray(
            wtpad.reshape(128, 2, 2, JW, KK)       # p, rp, rr, j', k
            .transpose(0, 1, 4, 2, 3)              # p, rp, k, rr, j'
            .reshape(128, WT_FREE))
        in_maps.append({"xbuf": X, "wtbuf": WT, "ident": identity})
    return in_maps


def _unpack_outputs(results) -> np.ndarray:
    out = np.empty((N, H, W, C), dtype=np.float32)
    for n in range(N_CORES):
        o = results[n]["out"].astype(np.float32)   # (2, 128, O_FREE)
        o = o.reshape(2, 128, C, 2, W)             # rp, p, c, rr, w
        # h = 4p + 2rp + rr
        out[n] = o.transpose(1, 0, 3, 4, 2).reshape(H, W, C)
    return out


def kernel(input_data: np.ndarray, weights: np.ndarray) -> np.ndarray:
    input_data = np.asarray(input_data, dtype=np.float32)
    weights = np.asarray(weights, dtype=np.float32)
    nc = _build_program()
    in_maps = _pack_inputs(input_data, weights)
    res = run_bass_kernel_spmd(nc, in_maps, list(range(N_CORES)))
    return _unpack_outputs(res.results)


if __name__ == "__main__":
    rng = np.random.default_rng(0)
    x = rng.standard_normal((N, H, W, C), dtype=np.float32)
    w = rng.standard_normal((N, H, W, KK), dtype=np.float32) * 0.1
    out = kernel(input_data=x, weights=w)

    xp = np.pad(x, ((0, 0), (2, 2), (2, 2), (0, 0)))
    exp = np.zeros_like(x)
    for k in range(KK):
        dh, dw = k // K, k % K
        exp += xp[:, dh:dh + H, dw:dw + W, :] * w[..., k:k + 1]
    diff = np.linalg.norm(out - exp) / np.linalg.norm(exp)
    print("out", out.shape, out.dtype, "rel err", diff)


# revision 3
# speedup vs baseline: 1.4047x; 1.4047x over previous
"""Trainium2 Bass kernel v3 for per-pixel (untied) local depthwise conv.

Problem: out[n,h,w,c] = sum_{dh,dw} in[n, h+dh-2, w+dw-2, c] * wt[n, h, w, dh*5+dw]
Shapes: in (8,512,512,3) f32, wt (8,512,512,25) f32, 'same' zero padding.

Design (one image per core, 8 cores):
  - The dw (column) shift is baked into per-tap weight planes on the HOST
    (content shifted, zero padded), so every DVE/Pool operand reads at its
    natural 4B-aligned position - no parity duplication of x.  The shift
    reappears as a column offset on the PE moving read, where it is free.
  - x stored once per core: [p, rr(8 halo rows), c, j(520)], 3.2MB fp16.
  - Output produced in TWO ROW-PASSES (rp = row-pair 2rp,2rp+1 of each
    partition): psum tile = 6 banks, bank = (c, rr) holding one full
    512-wide output row; each accumulation matmul is a contiguous 512-elem
    moving slice P[c, rr, dw:dw+512].
  - Products per (rp, tap): DVE tensor_mul [c,2,520] (TT is capped at 2x
    and 3 free dims on TRN2).  dh=1..4 on DVE, dh=0 on GPSIMD/Pool in
    parallel (psum accumulation order: dh 1,2,3,4 then 0).
  - Identity stationary loaded once: ldweights=False on all later matmuls.
  - Weight plane halves stream in consumption order; ~100KB SBUF total.
"""

import sys

sys.path.insert(0, "/opt/trn_rl_repo")

import numpy as np

import concourse.bass as bass
import concourse.mybir as mybir
from concourse.tile import TileContext
from concourse.bass_utils import run_bass_kernel_spmd

N, H, W, C, K = 8, 512, 512, 3, 5
KK = K * K
N_CORES = 8
RPP = 4                  # output rows per partition
HROWS = RPP + K - 1      # halo rows stored per partition (8)
JW = 520                 # padded row width (cols -2..517 at j-2)
X_FREE = HROWS * C * JW          # 12480 fp16 elems per partition
PJ = 516                 # weight plane width (union of dw shifts)
WT_HALF = 2 * PJ                 # 1032 elems per (rp, tap) weight chunk
WT_GROUP = K * WT_HALF           # one (rp, dh) group of 5 taps
WT_FREE = 2 * KK * WT_HALF       # 51600
P_FREE = C * 2 * PJ              # 3096 elems per (rp, tap) product
O_FREE = C * 2 * W               # 3072 out elems per partition per pass

USE_POOL = False         # Pool shares its SBUF port with DVE: net negative
SKIP_LDW = True          # delete repeat identity LDWEIGHTS from the IR


def _dedupe_identity_ldweights(nc):
    """Tile legalization splits every matmul into a standalone InstLdweights
    + non-self-loading InstMatmult.  All our matmuls share one identity
    stationary, so all but the first load per block are redundant: delete
    them and transplant their sync waits/updates onto the next PE
    instruction (the matmul).  _split_multi_waits legalizes any resulting
    multi-wait afterwards."""
    n_del = 0
    for f in nc.m.functions:
        for bb in f.blocks:
            seen_sig = None
            pending_waits, pending_updates = [], []
            new_insts = []
            for inst in bb.instructions:
                if isinstance(inst, mybir.InstLdweights):
                    sig = repr(inst.ins[0])
                    if seen_sig == sig:
                        si = inst.sync_info
                        if si is not None:
                            pending_waits.extend(si.on_wait or [])
                            pending_updates.extend(si.on_update or [])
                        n_del += 1
                        continue
                    seen_sig = sig
                elif (pending_waits or pending_updates) and \
                        inst.engine == mybir.EngineType.PE:
                    si = inst.sync_info
                    w = list(si.on_wait) if (si and si.on_wait) else []
                    u = list(si.on_update) if (si and si.on_update) else []
                    inst.sync_info = mybir.SyncInfo(
                        on_wait=pending_waits + w,
                        on_update=pending_updates + u,
                    )
                    pending_waits, pending_updates = [], []
                new_insts.append(inst)
            assert not pending_waits and not pending_updates
            bb.instructions = new_insts
    return n_del


def _split_multi_waits(nc):
    """This walrus build encodes at most ONE sync-wait per instruction;
    hoist extra waits onto single-wait NOPs on the same engine."""
    n_split = 0
    for f in nc.m.functions:
        for bb in f.blocks:
            new_insts = []
            changed = False
            for inst in bb.instructions:
                si = inst.sync_info
                waits = list(si.on_wait) if (si is not None and si.on_wait) else []
                if len(waits) > 1:
                    changed = True
                    for w in waits[:-1]:
                        nop = mybir.InstNoOp(
                            name=nc.get_next_instruction_name(),
                            engine=inst.engine,
                            sync_info=mybir.SyncInfo(on_wait=[w], on_update=[]),
                            bass_nofuse=True,
                        )
                        new_insts.append(nop)
                        n_split += 1
                    inst.sync_info = mybir.SyncInfo(
                        on_wait=[waits[-1]],
                        on_update=list(si.on_update) if si.on_update else [],
                    )
                new_insts.append(inst)
            if changed:
                bb.instructions = new_insts
    return n_split


_NC_CACHE = None


def _build_program():
    global _NC_CACHE
    if _NC_CACHE is not None:
        return _NC_CACHE

    fp16 = mybir.dt.float16
    f32 = mybir.dt.float32

    nc = bass.Bass("TRN2", target_bir_lowering=False, debug=False,
                   num_devices=N_CORES)
    xbuf = nc.dram_tensor("xbuf", [128, X_FREE], fp16, kind="ExternalInput").ap()
    # wtbuf[p, rp, k, rr, j']  (natural k order; dh0 first = Pool's units)
    wtbuf = nc.dram_tensor("wtbuf", [128, WT_FREE], fp16,
                           kind="ExternalInput").ap()
    ident = nc.dram_tensor("ident", [128, 128], fp16, kind="ExternalInput").ap()
    out = nc.dram_tensor("out", [2, 128, O_FREE], fp16,
                         kind="ExternalOutput").ap()

    mms = []
    with TileContext(nc) as tc:
        with (
            tc.tile_pool(name="xpool", bufs=1) as xpool,
            tc.tile_pool(name="wtpool", bufs=4) as wtpool,
            tc.tile_pool(name="ppool", bufs=8) as ppool,
            tc.tile_pool(name="outpool", bufs=2) as outpool,
            tc.tile_pool(name="psumpool", bufs=1, space="PSUM") as psumpool,
        ):
            id_t = xpool.tile([128, 128], fp16)
            nc.sync.dma_start(out=id_t[:], in_=ident[:])

            x_t = xpool.tile([128, X_FREE], fp16)
            XR = C * JW
            # pass 0 touches rows 0..5, pass 1 rows 2..7
            nc.sync.dma_start(out=x_t[:, 0:3 * XR], in_=xbuf[:, 0:3 * XR])
            nc.scalar.dma_start(out=x_t[:, 3 * XR:6 * XR],
                                in_=xbuf[:, 3 * XR:6 * XR])
            nc.sync.dma_start(out=x_t[:, 6 * XR:], in_=xbuf[:, 6 * XR:])

            xv = x_t[:].rearrange("p (r c j) -> p r c j", r=HROWS, c=C)

            # stream weight chunks in consumption order: one (rp, dh) group
            # of 5 taps per DMA (10.3KB/partition each)
            wt_tiles = {}
            for u in range(10):
                rp, dh = u // K, u % K
                wtile = wtpool.tile([128, WT_GROUP], fp16, tag="wt",
                                    name=f"wt_{rp}_{dh}")
                eng = nc.sync if u % 2 == 0 else nc.scalar
                eng.dma_start(out=wtile[:],
                              in_=wtbuf[:, u * WT_GROUP:(u + 1) * WT_GROUP])
                wt_tiles[(rp, dh)] = wtile

            def make_unit(rp, dh, dw):
                k = dh * K + dw
                p_t = ppool.tile([128, P_FREE], fp16, tag="p",
                                 name=f"p_{rp}_{k}")
                pv = p_t[:].rearrange("p (c r j) -> p c r j", c=C, r=2)
                r0 = dh + 2 * rp
                xs = xv[:, r0:r0 + 2, :, 0:PJ].transpose([0, 2, 1, 3])
                wk = (wt_tiles[(rp, dh)][:]
                      .rearrange("p (k r j) -> p k r j", k=K, r=2)
                      [:, dw][:, None, :, :]
                      .broadcast_to([128, C, 2, PJ]))
                nc.vector.tensor_mul(out=pv, in0=xs, in1=wk)
                return pv

            DH_ORDER = [0, 1, 2, 3, 4]

            for rp in range(2):
                ps_t = psumpool.tile([128, 6 * 512], f32, tag="ps",
                                     name=f"ps_{rp}")
                units = {}
                for di, dh in enumerate(DH_ORDER):
                    first_dh = di == 0
                    last_dh = di == K - 1
                    for dw in range(K):
                        if (dh, dw) not in units:
                            units[(dh, dw)] = make_unit(rp, dh, dw)
                        pv = units[(dh, dw)]
                        for c in range(C):
                            for rr in range(2):
                                bank = c * 2 + rr
                                mm = nc.tensor.matmul(
                                    ps_t[:, bank * 512:(bank + 1) * 512],
                                    id_t[:],
                                    pv[:, c, rr, dw:dw + W],
                                    start=(first_dh and dw == 0),
                                    stop=(last_dh and dw == K - 1),
                                )
                                mms.append(mm)

                o_t = outpool.tile([128, O_FREE], fp16)
                nc.scalar.copy(out=o_t[:], in_=ps_t[:])
                eng = nc.sync if rp == 0 else nc.scalar
                eng.dma_start(out=out[rp], in_=o_t[:])

    if SKIP_LDW:
        _dedupe_identity_ldweights(nc)

    _split_multi_waits(nc)
    _NC_CACHE = nc
    return nc


def _pack_inputs(input_data: np.ndarray, weights: np.ndarray):
    """Host-side layout + fp16 conversion into per-core SBUF-ready buffers."""
    xh = input_data.astype(np.float16)     # (N, H, W, C)
    wh = weights.astype(np.float16)        # (N, H, W, KK)

    identity = np.eye(128, dtype=np.float16)
    in_maps = []
    for n in range(N_CORES):
        # x: [p, rr, c, j]: image row 4p+rr-2, col j-2 (zero pad)
        canvas = np.zeros((C, H + 4, JW), dtype=np.float16)
        canvas[:, 2:2 + H, 2:2 + W] = xh[n].transpose(2, 0, 1)
        sw = np.lib.stride_tricks.sliding_window_view(canvas, HROWS, axis=1)
        sw = sw[:, ::RPP][:, :128]            # (C, 128, JW, 8)
        X = np.ascontiguousarray(
            sw.transpose(1, 3, 0, 2).reshape(128, X_FREE))

        # weight chunks: wtbuf[p, rp, dh, dw, rr, j'] = wt[4p+2rp+rr, j'-dw, k]
        wtpad = np.zeros((H, PJ, KK), dtype=np.float16)
        for dw in range(K):
            wtpad[:, dw:dw + W, dw::K] = wh[n][:, :, dw::K]
        WT = np.ascontiguousarray(
            wtpad.reshape(128, 2, 2, PJ, KK)       # p, rp, rr, j', k
            .transpose(0, 1, 4, 2, 3)              # p, rp, k, rr, j'
            .reshape(128, WT_FREE))
        in_maps.append({"xbuf": X, "wtbuf": WT, "ident": identity})
    return in_maps


def _unpack_outputs(results) -> np.ndarray:
    out = np.empty((N, H, W, C), dtype=np.float32)
    for n in range(N_CORES):
        o = results[n]["out"].astype(np.float32)   # (2, 128, O_FREE)
        o = o.reshape(2, 128, C, 2, W)             # rp, p, c, rr, w
        # h = 4p + 2rp + rr
        out[n] = o.transpose(1, 0, 3, 4, 2).reshape(H, W, C)
    return out


def kernel(input_data: np.ndarray, weights: np.ndarray) -> np.ndarray:
    input_data = np.asarray(input_data, dtype=np.float32)
    weights = np.asarray(weights, dtype=np.float32)
    nc = _build_program()
    in_maps = _pack_inputs(input_data, weights)
    res = run_bass_kernel_spmd(nc, in_maps, list(range(N_CORES)))
    return _unpack_outputs(res.results)


if __name__ == "__main__":
    rng = np.random.default_rng(0)
    x = rng.standard_normal((N, H, W, C), dtype=np.float32)
    w = rng.standard_normal((N, H, W, KK), dtype=np.float32) * 0.1
    out = kernel(input_data=x, weights=w)

    xp = np.pad(x, ((0, 0), (2, 2), (2, 2), (0, 0)))
    exp = np.zeros_like(x)
    for k in range(KK):
        dh, dw = k // K, k % K
        exp += xp[:, dh:dh + H, dw:dw + W, :] * w[..., k:k + 1]
    diff = np.linalg.norm(out - exp) / np.linalg.norm(exp)
    print("out", out.shape, out.dtype, "rel err", diff)
